# revision 1
# baseline (speedup 1.0000x reference)
"""CoLAttention Trainium2 kernel (8-core data-parallel SPMD).

Computes, per batch b:
    Q   = x @ W_Q.T + b_Q
    A   = softmax((Q @ C_K) / sqrt(D), axis=-1) * mask[..., None]
    out = A @ C_V.T

Algebraic restructure done on host (exact up to fp rounding):
    S    = x @ M + biasT          where  M = (W_Q.T @ C_K)/sqrt(D)  [D, A]
                                          biasT = (b_Q @ C_K)/sqrt(D)  [A]
    out  = (exp(S) @ C_V.T) * (mask / sum_a exp(S))[:, None]
(logits |S| < ~0.3 for these input stats, so no max-subtraction is needed;
the softmax denominator is applied after the second matmul by linearity.
The denominator uses the same rounded exp values as mm2, so the softmax
normalization is exact w.r.t. the rounded weights.)

Device dataflow per core (one batch of x, host-cast to bf16 [4096, 1024]):
  - one xbar DMA-transpose per 512-row strip, straight from DRAM:
    [512 l, 1024 d] -> SBUF [128 d, 8 k, 512 l]  (k-major d-chunks),
    alternating between the two HWDGE engines (SP / ACT) so the blocking
    ucode transpose doesn't serialize on one sequencer
  - mm1: S.T [64, 512] = sum_k Mb_k.T @ xT_k   (bf16, N=512)
  - ACT: expT = Exp(S.T + biasT)  (per-partition bias fused)
  - rowsums: expT_slice.T @ ones -> [128, 2] psum (N=2; fp32 accumulate)
  - mm2: out [128, 512] = expT_slice.T @ C_V.T chunk
  - scale by (mask * 1/rowsum) per-partition into a [128, 4, 1024] strip
    buffer (alternating DVE/ACT), stored with ONE DMA per strip
"""

import math
import os
import sys

import numpy as np

for _p in ("/opt/trn_rl_repo",):
    if _p not in sys.path and os.path.isdir(_p):
        sys.path.insert(0, _p)

B, L, D, A = 8, 4096, 1024, 64
N_CORES = 8
P = 128  # partitions
SL = 512  # l-strip length
NSTRIP = L // SL  # 8
NJ = SL // P  # 4 l-subtiles per strip
NK = D // P  # 8 d-chunks
NE = D // SL  # 2 e-chunks of the output row

OUT_BF16 = True  # store output as bf16 (halves store traffic)


def _build_nc():
    import concourse.bass as bass
    import concourse.tile as tile
    from concourse import bacc, mybir

    f32 = mybir.dt.float32
    bf16 = mybir.dt.bfloat16
    EXP = mybir.ActivationFunctionType.Exp
    out_dt = bf16 if OUT_BF16 else f32

    nc = bacc.Bacc(
        "TRN2",
        target_bir_lowering=False,
        debug=False,
        enable_asserts=False,
        num_devices=N_CORES,
    )

    x_ap = nc.dram_tensor("x", [L // 2, 2 * D], bf16, kind="ExternalInput").ap()
    # packed constants: cb (bf16) = [mw(p,(k,a)) | cvt rows 0-63 | ones rows 0-63]
    #                   cf (f32)  = [maskt | bias rows 0-63]
    CB_W = NK * A + D + 2
    cb_ap = nc.dram_tensor("cb", [P, CB_W], bf16, kind="ExternalInput").ap()
    cf_ap = nc.dram_tensor("cf", [P, L // P + 1], f32, kind="ExternalInput").ap()
    out_ap = nc.dram_tensor("out", [L, D], out_dt, kind="ExternalOutput").ap()

    out_r = out_ap.rearrange("(s half p par) d -> s p par half d", half=2, p=P, par=2)

    with tile.TileContext(nc) as tc:
        with (
            tc.tile_pool(name="consts", bufs=1) as consts,
            tc.tile_pool(name="xt", bufs=8) as xt_pool,
            tc.tile_pool(name="st", bufs=2, space="PSUM") as st_pool,
            tc.tile_pool(name="rs", bufs=2, space="PSUM") as rs_pool,
            tc.tile_pool(name="op", bufs=3, space="PSUM") as op_pool,
            tc.tile_pool(name="wu", bufs=1, space="PSUM") as wu_pool,
            tc.tile_pool(name="et", bufs=2) as et_pool,
            tc.tile_pool(name="sc", bufs=4) as sc_pool,
            tc.tile_pool(name="ob", bufs=8) as ob_pool,
        ):
            # HAM warm-up: ~36 junk matmuls with no DMA dependencies fill the
            # PE from right after the entry barrier until the first transpose
            # lands, unthrottling the PE clock (1.2 -> 2.4 GHz) before real
            # work and keeping it warm.
            wu_sb = consts.tile([P, SL], bf16)
            nc.vector.memset(wu_sb, 1.0)
            wu_ps = wu_pool.tile([P, SL], f32)
            for _ in range(36):
                nc.tensor.matmul(
                    wu_ps, lhsT=wu_sb[:, 0:P], rhs=wu_sb, start=True, stop=True
                )

            cb_sb = consts.tile([P, CB_W], bf16)
            nc.sync.dma_start(out=cb_sb, in_=cb_ap)
            cf_sb = consts.tile([P, L // P + 1], f32)
            nc.sync.dma_start(out=cf_sb, in_=cf_ap)
            mw_sb = cb_sb[:, 0 : NK * A].rearrange("p (k a) -> p k a", k=NK)
            cvt_sb = cb_sb[0:A, NK * A : NK * A + D]
            ones_sb = cb_sb[0:A, NK * A + D : NK * A + D + 2]
            maskt_sb = cf_sb[:, 0 : L // P]
            bias_sb = cf_sb[0:A, L // P : L // P + 1]

            # Phase 1: all xbar transposes back-to-back on the SP sequencer
            # (a single xbar-mode phase -> no per-strip mode-switch drains),
            # reading the row-pair view [256, 2048] per strip:
            #   xt[p, par, k, t] = x[512*s + 2*t + par, 128*k + p]
            xts = []
            t_insts = []
            for s in range(NSTRIP):
                xt_t = xt_pool.tile([P, 2, NK, SL // 2], bf16, tag="xt")
                ti = nc.sync.dma_start(
                    out=xt_t,
                    in_=x_ap[s * (SL // 2) : (s + 1) * (SL // 2), :],
                    transpose=True,
                )
                xts.append(xt_t)
                t_insts.append(ti)

            for s in range(NSTRIP):
                xt_t = xts[s]
                # mm1: S.T [64, 512] accumulated over 8 d-chunks (bf16)
                st = st_pool.tile([A, SL], f32, tag="st")
                for k in range(NK):
                    nc.tensor.matmul(
                        st,
                        lhsT=mw_sb[:, k, :],
                        rhs=xt_t[:, :, k, :],
                        start=(k == 0),
                        stop=(k == NK - 1),
                    )

                # expT = exp(S.T + bias)
                et = et_pool.tile([A, SL], bf16, tag="et")
                nc.scalar.activation(et, st, EXP, bias=bias_sb)

                ob = ob_pool.tile([P, NJ, D], out_dt, tag="ob")
                for j in range(NJ):
                    lcol = s * NJ + j  # global l-subtile index (0..31)
                    rs = rs_pool.tile([P, 2], f32, tag="rs")
                    nc.tensor.matmul(
                        rs,
                        lhsT=et[:, j * P : (j + 1) * P],
                        rhs=ones_sb,
                        start=True,
                        stop=True,
                    )
                    sc = sc_pool.tile([P, 1], f32, tag="sc")
                    nc.vector.reciprocal(sc, rs[:, 0:1])
                    scm = sc_pool.tile([P, 1], f32, tag="scm")
                    nc.vector.tensor_mul(scm, sc, maskt_sb[:, lcol : lcol + 1])

                    for e in range(NE):
                        op = op_pool.tile([P, SL], f32, tag="op")
                        nc.tensor.matmul(
                            op,
                            lhsT=et[:, j * P : (j + 1) * P],
                            rhs=cvt_sb[:, e * SL : (e + 1) * SL],
                            start=True,
                            stop=True,
                        )
                        dst = ob[:, j, e * SL : (e + 1) * SL]
                        if (j * NE + e) % 2:
                            nc.scalar.mul(dst, op, scm)
                        else:
                            nc.vector.tensor_scalar_mul(dst, op, scm)
                # one store per strip (interleaved rows via strided AP);
                # ordered after the last transpose to avoid xbar-mode flips
                st_i = nc.sync.dma_start(out=out_r[s], in_=ob)
                tile.add_dep_helper(
                    st_i.ins, t_insts[-1].ins,
                    reason="keep copy-mode stores after the xbar phase",
                )

    nc.compile()
    return nc


_NC_CACHE = None


def _get_nc():
    global _NC_CACHE
    if _NC_CACHE is None:
        _NC_CACHE = _build_nc()
    return _NC_CACHE


def _host_inputs(x, mask, W_Q, b_Q, C_K, C_V):
    """Per-core input maps for run_bass_kernel_spmd."""
    import ml_dtypes

    bf = ml_dtypes.bfloat16
    inv_sqrt_d = np.float32(1.0 / math.sqrt(D))
    mw = (W_Q.T.astype(np.float32) @ C_K.astype(np.float32)) * inv_sqrt_d
    mw_bf = np.ascontiguousarray(mw.astype(bf))  # [D, A]
    cvt_bf = np.ascontiguousarray(C_V.T.astype(bf))  # [A, D]
    biasT = ((b_Q.astype(np.float32) @ C_K.astype(np.float32)) * inv_sqrt_d).reshape(
        A, 1
    )
    biasT = np.ascontiguousarray(biasT, dtype=np.float32)
    ones = np.ones((A, 2), dtype=bf)

    cb = np.zeros((P, NK * A + D + 2), dtype=bf)
    cb[:, 0 : NK * A] = mw_bf.reshape(NK, P, A).transpose(1, 0, 2).reshape(P, NK * A)
    cb[0:A, NK * A : NK * A + D] = cvt_bf
    cb[0:A, NK * A + D :] = ones.astype(bf)
    in_maps = []
    for c in range(N_CORES):
        # maskt[p, 4*s + jp] = mask[c, l] with the row-pair permutation
        # l = 512*s + 256*(jp%2) + 2*p + jp//2
        mf = mask[c].astype(np.float32)
        maskt = np.empty((P, L // P), dtype=np.float32)
        pidx = np.arange(P)
        for s_ in range(NSTRIP):
            for jp in range(NJ):
                l_idx = 512 * s_ + 256 * (jp % 2) + 2 * pidx + (jp // 2)
                maskt[:, 4 * s_ + jp] = mf[l_idx]
        cf = np.zeros((P, L // P + 1), dtype=np.float32)
        cf[:, 0 : L // P] = maskt
        cf[0:A, L // P] = biasT[:, 0]
        in_maps.append(
            {
                "x": np.ascontiguousarray(x[c].astype(bf)).reshape(L // 2, 2 * D),
                "cb": cb,
                "cf": cf,
            }
        )
    return in_maps


def kernel(**inputs):
    x = np.asarray(inputs["x"], dtype=np.float32)
    mask = np.asarray(inputs["mask"])
    W_Q = np.asarray(inputs["W_Q"], dtype=np.float32)
    b_Q = np.asarray(inputs["b_Q"], dtype=np.float32)
    C_K = np.asarray(inputs["C_K"], dtype=np.float32)
    C_V = np.asarray(inputs["C_V"], dtype=np.float32)

    from concourse.bass_utils import run_bass_kernel_spmd

    nc = _get_nc()
    in_maps = _host_inputs(x, mask, W_Q, b_Q, C_K, C_V)
    res = run_bass_kernel_spmd(nc, in_maps, core_ids=list(range(N_CORES)))
    results = res.results if hasattr(res, "results") else res
    out = np.stack(
        [np.asarray(results[c]["out"]).astype(np.float32) for c in range(N_CORES)],
        axis=0,
    )
    return np.ascontiguousarray(out, dtype=np.float32)



# revision 3
# speedup vs baseline: 1.0708x; 1.0708x over previous
"""CoLAttention Trainium2 kernel (8-core data-parallel SPMD).

Computes, per batch b:
    Q   = x @ W_Q.T + b_Q
    A   = softmax((Q @ C_K) / sqrt(D), axis=-1) * mask[..., None]
    out = A @ C_V.T

Algebraic restructure done on host (exact up to fp rounding):
    S    = x @ M + biasT          where  M = (W_Q.T @ C_K)/sqrt(D)  [D, A]
                                          biasT = (b_Q @ C_K)/sqrt(D)  [A]
    out  = (exp(S) @ C_V.T) * (mask / sum_a exp(S))[:, None]
(logits |S| < ~0.3 for these input stats, so no max-subtraction is needed;
the softmax denominator is applied after the second matmul by linearity.
The denominator uses the same rounded exp values as mm2, so the softmax
normalization is exact w.r.t. the rounded weights.)

Device dataflow per core (one batch of x, host-cast to bf16 [4096, 1024]):
  - one xbar DMA-transpose per 512-row strip, straight from DRAM:
    [512 l, 1024 d] -> SBUF [128 d, 8 k, 512 l]  (k-major d-chunks),
    alternating between the two HWDGE engines (SP / ACT) so the blocking
    ucode transpose doesn't serialize on one sequencer
  - mm1: S.T [64, 512] = sum_k Mb_k.T @ xT_k   (bf16, N=512)
  - ACT: expT = Exp(S.T + biasT)  (per-partition bias fused)
  - rowsums: expT_slice.T @ ones -> [128, 2] psum (N=2; fp32 accumulate)
  - mm2: out [128, 512] = expT_slice.T @ C_V.T chunk
  - scale by (mask * 1/rowsum) per-partition into a [128, 4, 1024] strip
    buffer (alternating DVE/ACT), stored with ONE DMA per strip
"""

import math
import os
import sys

import numpy as np

for _p in ("/opt/trn_rl_repo",):
    if _p not in sys.path and os.path.isdir(_p):
        sys.path.insert(0, _p)

B, L, D, A = 8, 4096, 1024, 64
N_CORES = 8
P = 128  # partitions
SL = 512  # l-strip length
NSTRIP = L // SL  # 8
NJ = SL // P  # 4 l-subtiles per strip
NK = D // P  # 8 d-chunks
NE = D // SL  # 2 e-chunks of the output row

OUT_BF16 = True  # store output as bf16 (halves store traffic)


def _build_nc():
    import concourse.bass as bass
    import concourse.tile as tile
    from concourse import bacc, mybir

    f32 = mybir.dt.float32
    bf16 = mybir.dt.bfloat16
    EXP = mybir.ActivationFunctionType.Exp
    out_dt = bf16 if OUT_BF16 else f32

    nc = bacc.Bacc(
        "TRN2",
        target_bir_lowering=False,
        debug=False,
        enable_asserts=False,
        num_devices=N_CORES,
    )

    x_ap = nc.dram_tensor("x", [L // 2, 2 * D], bf16, kind="ExternalInput").ap()
    # packed constants: cb (bf16) = [mw(p,(k,a)) | cvt rows 0-63 | ones rows 0-63]
    #                   cf (f32)  = [maskt | bias rows 0-63]
    CB_W = NK * A + D + 2
    cb_ap = nc.dram_tensor("cb", [P, CB_W], bf16, kind="ExternalInput").ap()
    cf_ap = nc.dram_tensor("cf", [P, L // P + 1], f32, kind="ExternalInput").ap()
    out_ap = nc.dram_tensor("out", [L, D], out_dt, kind="ExternalOutput").ap()

    out_r = out_ap.rearrange("(s half p par) d -> s p par half d", half=2, p=P, par=2)

    with tile.TileContext(nc) as tc:
        with (
            tc.tile_pool(name="consts", bufs=1) as consts,
            tc.tile_pool(name="xt", bufs=8) as xt_pool,
            tc.tile_pool(name="st", bufs=2, space="PSUM") as st_pool,
            tc.tile_pool(name="rs", bufs=2, space="PSUM") as rs_pool,
            tc.tile_pool(name="op", bufs=4, space="PSUM") as op_pool,
            tc.tile_pool(name="et", bufs=2) as et_pool,
            tc.tile_pool(name="sc", bufs=4) as sc_pool,
            tc.tile_pool(name="ob", bufs=8) as ob_pool,
        ):
            # const loads go on the gpsimd SW-DGE queue so the SP queue's
            # first instruction is the strip-0 transpose (no copy->xbar
            # mode flip in front of it)
            cb_sb = consts.tile([P, CB_W], bf16)
            nc.gpsimd.dma_start(out=cb_sb, in_=cb_ap)
            cf_sb = consts.tile([P, L // P + 1], f32)
            nc.gpsimd.dma_start(out=cf_sb, in_=cf_ap)
            mw_sb = cb_sb[:, 0 : NK * A].rearrange("p (k a) -> p k a", k=NK)
            cvt_sb = cb_sb[0:A, NK * A : NK * A + D]
            ones_sb = cb_sb[0:A, NK * A + D : NK * A + D + 2]
            maskt_sb = cf_sb[:, 0 : L // P]
            bias_sb = cf_sb[0:A, L // P : L // P + 1]

            # Phase 1: all xbar transposes back-to-back on the SP sequencer
            # (a single xbar-mode phase -> no per-strip mode-switch drains),
            # reading the row-pair view [256, 2048] per strip:
            #   xt[p, par, k, t] = x[512*s + 2*t + par, 128*k + p]
            xts = []
            t_insts = []
            for s in range(NSTRIP):
                xt_t = xt_pool.tile([P, 2, NK, SL // 2], bf16, tag="xt")
                ti = nc.sync.dma_start(
                    out=xt_t,
                    in_=x_ap[s * (SL // 2) : (s + 1) * (SL // 2), :],
                    transpose=True,
                )
                xts.append(xt_t)
                t_insts.append(ti)

            for s in range(NSTRIP):
                xt_t = xts[s]
                # mm1: S.T [64, 512] accumulated over 8 d-chunks (bf16)
                st = st_pool.tile([A, SL], f32, tag="st")
                for k in range(NK):
                    nc.tensor.matmul(
                        st,
                        lhsT=mw_sb[:, k, :],
                        rhs=xt_t[:, :, k, :],
                        start=(k == 0),
                        stop=(k == NK - 1),
                    )

                # expT = exp(S.T + bias)
                et = et_pool.tile([A, SL], bf16, tag="et")
                nc.scalar.activation(et, st, EXP, bias=bias_sb)

                ob = ob_pool.tile([P, NJ, D], out_dt, tag="ob")
                for j in range(NJ):
                    lcol = s * NJ + j  # global l-subtile index (0..31)
                    rs = rs_pool.tile([P, 1], f32, tag="rs")
                    nc.tensor.matmul(
                        rs,
                        lhsT=et[:, j * P : (j + 1) * P],
                        rhs=ones_sb[:, 0:1],
                        start=True,
                        stop=True,
                    )
                    sc = sc_pool.tile([P, 1], f32, tag="sc")
                    nc.vector.reciprocal(sc, rs[:, 0:1])
                    scm = sc_pool.tile([P, 1], f32, tag="scm")
                    nc.vector.tensor_mul(scm, sc, maskt_sb[:, lcol : lcol + 1])

                    for e in range(NE):
                        op = op_pool.tile([P, SL], f32, tag="op")
                        nc.tensor.matmul(
                            op,
                            lhsT=et[:, j * P : (j + 1) * P],
                            rhs=cvt_sb[:, e * SL : (e + 1) * SL],
                            start=True,
                            stop=True,
                        )
                        dst = ob[:, j, e * SL : (e + 1) * SL]
                        if (j * NE + e) % 2:
                            nc.scalar.mul(dst, op, scm)
                        else:
                            nc.vector.tensor_scalar_mul(dst, op, scm)
                # one store per strip (interleaved rows via strided AP);
                # ordered after the last transpose to avoid xbar-mode flips
                st_i = nc.sync.dma_start(out=out_r[s], in_=ob)
                tile.add_dep_helper(
                    st_i.ins, t_insts[-1].ins,
                    reason="keep copy-mode stores after the xbar phase",
                )

    nc.compile()
    return nc


_NC_CACHE = None


def _get_nc():
    global _NC_CACHE
    if _NC_CACHE is None:
        _NC_CACHE = _build_nc()
    return _NC_CACHE


def _host_inputs(x, mask, W_Q, b_Q, C_K, C_V):
    """Per-core input maps for run_bass_kernel_spmd."""
    import ml_dtypes

    bf = ml_dtypes.bfloat16
    inv_sqrt_d = np.float32(1.0 / math.sqrt(D))
    mw = (W_Q.T.astype(np.float32) @ C_K.astype(np.float32)) * inv_sqrt_d
    mw_bf = np.ascontiguousarray(mw.astype(bf))  # [D, A]
    cvt_bf = np.ascontiguousarray(C_V.T.astype(bf))  # [A, D]
    biasT = ((b_Q.astype(np.float32) @ C_K.astype(np.float32)) * inv_sqrt_d).reshape(
        A, 1
    )
    biasT = np.ascontiguousarray(biasT, dtype=np.float32)
    ones = np.ones((A, 2), dtype=bf)

    cb = np.zeros((P, NK * A + D + 2), dtype=bf)
    cb[:, 0 : NK * A] = mw_bf.reshape(NK, P, A).transpose(1, 0, 2).reshape(P, NK * A)
    cb[0:A, NK * A : NK * A + D] = cvt_bf
    cb[0:A, NK * A + D :] = ones.astype(bf)
    in_maps = []
    for c in range(N_CORES):
        # maskt[p, 4*s + jp] = mask[c, l] with the row-pair permutation
        # l = 512*s + 256*(jp%2) + 2*p + jp//2
        mf = mask[c].astype(np.float32)
        maskt = np.empty((P, L // P), dtype=np.float32)
        pidx = np.arange(P)
        for s_ in range(NSTRIP):
            for jp in range(NJ):
                l_idx = 512 * s_ + 256 * (jp % 2) + 2 * pidx + (jp // 2)
                maskt[:, 4 * s_ + jp] = mf[l_idx]
        cf = np.zeros((P, L // P + 1), dtype=np.float32)
        cf[:, 0 : L // P] = maskt
        cf[0:A, L // P] = biasT[:, 0]
        in_maps.append(
            {
                "x": np.ascontiguousarray(x[c].astype(bf)).reshape(L // 2, 2 * D),
                "cb": cb,
                "cf": cf,
            }
        )
    return in_maps


def kernel(**inputs):
    x = np.asarray(inputs["x"], dtype=np.float32)
    mask = np.asarray(inputs["mask"])
    W_Q = np.asarray(inputs["W_Q"], dtype=np.float32)
    b_Q = np.asarray(inputs["b_Q"], dtype=np.float32)
    C_K = np.asarray(inputs["C_K"], dtype=np.float32)
    C_V = np.asarray(inputs["C_V"], dtype=np.float32)

    from concourse.bass_utils import run_bass_kernel_spmd

    nc = _get_nc()
    in_maps = _host_inputs(x, mask, W_Q, b_Q, C_K, C_V)
    res = run_bass_kernel_spmd(nc, in_maps, core_ids=list(range(N_CORES)))
    results = res.results if hasattr(res, "results") else res
    out = np.stack(
        [np.asarray(results[c]["out"]).astype(np.float32) for c in range(N_CORES)],
        axis=0,
    )
    return np.ascontiguousarray(out, dtype=np.float32)



# revision 6
# speedup vs baseline: 1.2611x; 1.1777x over previous
"""CoLAttention Trainium2 kernel (8-core data-parallel SPMD).

Computes, per batch b:
    Q   = x @ W_Q.T + b_Q
    A   = softmax((Q @ C_K) / sqrt(D), axis=-1) * mask[..., None]
    out = A @ C_V.T

Algebraic restructure done on host (exact up to fp rounding):
    S    = x @ M + biasT          where  M = (W_Q.T @ C_K)/sqrt(D)  [D, A]
                                          biasT = (b_Q @ C_K)/sqrt(D)  [A]
    out  = (exp(S) @ C_V.T) * (mask / sum_a exp(S))[:, None]
(logits |S| < ~0.3 for these input stats, so no max-subtraction is needed;
the softmax denominator is applied after the second matmul by linearity.
The denominator uses the same rounded exp values as mm2, so the softmax
normalization is exact w.r.t. the rounded weights.)

fp8 DoubleRow mm1: x and M are quantized to e4m3 (M pre-scaled by 512; the
exp activation applies 1/512).  Host packs adjacent-d pairs of fp8 x into
one 16-bit unit, so a single xbar DMA transpose both halves the input
stream bytes and lands the pairs as the DoubleRow k-tile dim:
  SBUF partition p, free (c, 2t+b) = fp8 x[512s+t, 256c+2p+b]
mm1 then runs perf_mode=DoubleRow (contraction 256/instr, 0.5 cyc/row):
  st[a, t] (x512) = sum_c sum_p sum_b mwq[256c+2p+b, a] * x8[l, 256c+2p+b]

Device dataflow per core:
  - cb/cf const loads then 8 per-strip xbar transposes, all on the SP
    queue in emission order (one copy->xbar mode flip; stores are kept
    after the last transpose because the xbar mode is global to the DMA
    fabric)
  - mm1: S.T halves [64, 256] in fp8 DoubleRow, 4 c-chunks each
  - ACT: expT = Exp(S.T * (1/512) + biasT)  (bias + descale fused)
  - rowsums: expT_slice.T @ ones -> [128, 1] psum (fp32 accumulate)
  - mm2: out [128, 512] = expT_slice.T @ C_V.T chunk  (bf16)
  - scale by (mask * 1/rowsum) per-partition into a [128, 4, 1024] strip
    buffer (alternating DVE/ACT), stored with ONE DMA per strip
"""

import math
import os
import sys

import numpy as np

for _p in ("/opt/trn_rl_repo",):
    if _p not in sys.path and os.path.isdir(_p):
        sys.path.insert(0, _p)

B, L, D, A = 8, 4096, 1024, 64
N_CORES = 8
P = 128  # partitions
SL = 512  # l-strip length
NSTRIP = L // SL  # 8
NJ = SL // P  # 4 l-subtiles per strip
NC = D // 256  # 4 DoubleRow contraction chunks (256 d each)
NE = D // SL  # 2 e-chunks of the output row

MW_SCALE = 512.0  # fp8 pre-scale on M; undone in the exp activation

OUT_BF16 = True  # store output as bf16 (halves store traffic)

# cb (bf16-typed) layout: [mwdr bytes (256 bf16 cols) | cvt (1024) | ones (2)]
MW_COLS = 256  # 512 fp8 bytes viewed as 256 bf16 columns
CB_W = MW_COLS + D + 2


def _build_nc():
    import concourse.bass as bass
    import concourse.tile as tile
    from concourse import bacc, mybir

    f32 = mybir.dt.float32
    bf16 = mybir.dt.bfloat16
    f8 = mybir.dt.float8e4
    EXP = mybir.ActivationFunctionType.Exp
    DR = mybir.MatmulPerfMode.DoubleRow
    out_dt = bf16 if OUT_BF16 else f32

    nc = bacc.Bacc(
        "TRN2",
        target_bir_lowering=False,
        debug=False,
        enable_asserts=False,
        num_devices=N_CORES,
    )

    # x packed as fp8 d-pairs in 16-bit units: [L, D/2] "bf16"
    x_ap = nc.dram_tensor("x", [L, D // 2], bf16, kind="ExternalInput").ap()
    cb_ap = nc.dram_tensor("cb", [P, CB_W], bf16, kind="ExternalInput").ap()
    cf_ap = nc.dram_tensor("cf", [P, L // P + 1], f32, kind="ExternalInput").ap()
    out_ap = nc.dram_tensor("out", [L, D], out_dt, kind="ExternalOutput").ap()

    # strip s, subtile j, partition p: l = 512s + 128j + p
    out_r = out_ap.rearrange("(s j p) d -> s p j d", j=NJ, p=P)

    with tile.TileContext(nc) as tc:
        with (
            tc.tile_pool(name="consts", bufs=1) as consts,
            tc.tile_pool(name="xt", bufs=8) as xt_pool,
            tc.tile_pool(name="st", bufs=2, space="PSUM") as st_pool,
            tc.tile_pool(name="rs", bufs=2, space="PSUM") as rs_pool,
            tc.tile_pool(name="op", bufs=4, space="PSUM") as op_pool,
            tc.tile_pool(name="et", bufs=2) as et_pool,
            tc.tile_pool(name="sc", bufs=4) as sc_pool,
            tc.tile_pool(name="ob", bufs=8) as ob_pool,
        ):
            cb_sb = consts.tile([P, CB_W], bf16)
            nc.sync.dma_start(out=cb_sb, in_=cb_ap)
            cf_sb = consts.tile([P, L // P + 1], f32)
            nc.sync.dma_start(out=cf_sb, in_=cf_ap)
            # mwdr[p, c, b, a] = fp8(512 * M[256c + 2p + b, a])
            mwdr_sb = (
                cb_sb[:, 0:MW_COLS]
                .bitcast(f8)
                .rearrange("p (c b a) -> p c b a", c=NC, b=2)
            )
            cvt_sb = cb_sb[0:A, MW_COLS : MW_COLS + D]
            ones_sb = cb_sb[0:A, MW_COLS + D : MW_COLS + D + 2]
            maskt_sb = cf_sb[:, 0 : L // P]
            bias_sb = cf_sb[0:A, L // P : L // P + 1]

            # Phase 1: all xbar transposes back-to-back on the SP sequencer
            # (a single xbar-mode phase -> no per-strip mode-switch drains):
            #   xt[p, c, t] = xpair[512s + t, 128c + p]
            xts = []
            t_insts = []
            for s in range(NSTRIP):
                xt_t = xt_pool.tile([P, NC, SL], bf16, tag="xt")
                ti = nc.sync.dma_start(
                    out=xt_t,
                    in_=x_ap[s * SL : (s + 1) * SL, :],
                    transpose=True,
                )
                xts.append(xt_t)
                t_insts.append(ti)

            for s in range(NSTRIP):
                xt8 = xts[s].bitcast(f8)  # [P, NC, 2*SL] fp8
                # mm1: S.T halves [64, 256], fp8 DoubleRow, accumulated
                # over 4 c-chunks (contraction 256 d per instruction)
                sts = []
                for h in range(2):
                    sth = st_pool.tile([A, SL // 2], f32, tag="st")
                    sts.append(sth)
                for c in range(NC):
                    full = xt8[:, c].rearrange("p (l b) -> p b l", b=2)
                    for h in range(2):
                        nc.tensor.matmul(
                            sts[h],
                            lhsT=mwdr_sb[:, c],
                            rhs=full[:, :, h * (SL // 2) : (h + 1) * (SL // 2)],
                            start=(c == 0),
                            stop=(c == NC - 1),
                            perf_mode=DR,
                        )

                # expT = exp(S.T/512 + bias)
                et = et_pool.tile([A, SL], bf16, tag="et")
                for h in range(2):
                    nc.scalar.activation(
                        et[:, h * (SL // 2) : (h + 1) * (SL // 2)],
                        sts[h],
                        EXP,
                        bias=bias_sb,
                        scale=1.0 / MW_SCALE,
                    )

                ob = ob_pool.tile([P, NJ, D], out_dt, tag="ob")
                for j in range(NJ):
                    lcol = s * NJ + j  # global l-subtile index (0..31)
                    rs = rs_pool.tile([P, 1], f32, tag="rs")
                    nc.tensor.matmul(
                        rs,
                        lhsT=et[:, j * P : (j + 1) * P],
                        rhs=ones_sb[:, 0:1],
                        start=True,
                        stop=True,
                    )
                    sc = sc_pool.tile([P, 1], f32, tag="sc")
                    nc.vector.reciprocal(sc, rs[:, 0:1])
                    scm = sc_pool.tile([P, 1], f32, tag="scm")
                    nc.vector.tensor_mul(scm, sc, maskt_sb[:, lcol : lcol + 1])

                    for e in range(NE):
                        op = op_pool.tile([P, SL], f32, tag="op")
                        nc.tensor.matmul(
                            op,
                            lhsT=et[:, j * P : (j + 1) * P],
                            rhs=cvt_sb[:, e * SL : (e + 1) * SL],
                            start=True,
                            stop=True,
                        )
                        dst = ob[:, j, e * SL : (e + 1) * SL]
                        if (j * NE + e) % 2:
                            nc.scalar.mul(dst, op, scm)
                        else:
                            nc.vector.tensor_scalar_mul(dst, op, scm)
                # one store per strip (interleaved rows via strided AP);
                # ordered after the last transpose to avoid xbar-mode flips
                st_i = nc.sync.dma_start(out=out_r[s], in_=ob)
                tile.add_dep_helper(
                    st_i.ins, t_insts[-1].ins,
                    reason="keep copy-mode stores after the xbar phase",
                )

    nc.compile()
    return nc


_NC_CACHE = None


def _get_nc():
    global _NC_CACHE
    if _NC_CACHE is None:
        _NC_CACHE = _build_nc()
    return _NC_CACHE


def _host_inputs(x, mask, W_Q, b_Q, C_K, C_V):
    """Per-core input maps for run_bass_kernel_spmd."""
    import ml_dtypes

    bf = ml_dtypes.bfloat16
    f8 = ml_dtypes.float8_e4m3fn
    inv_sqrt_d = np.float32(1.0 / math.sqrt(D))
    mw = (W_Q.T.astype(np.float32) @ C_K.astype(np.float32)) * inv_sqrt_d
    mwq = np.ascontiguousarray((mw * MW_SCALE).astype(f8))  # [D, A]
    # mwdr[p, c, b, a] = mwq[256c + 2p + b, a]
    mwdr = mwq.reshape(NC, P, 2, A).transpose(1, 0, 2, 3)
    mwdr_b = (
        np.ascontiguousarray(mwdr).reshape(P, 2 * NC * A).view(np.uint16).view(bf)
    )  # [P, 256]
    cvt_bf = np.ascontiguousarray(C_V.T.astype(bf))  # [A, D]
    biasT = ((b_Q.astype(np.float32) @ C_K.astype(np.float32)) * inv_sqrt_d).reshape(
        A, 1
    )
    biasT = np.ascontiguousarray(biasT, dtype=np.float32)
    ones = np.ones((A, 2), dtype=bf)

    cb = np.zeros((P, CB_W), dtype=bf)
    cb[:, 0:MW_COLS] = mwdr_b
    cb[0:A, MW_COLS : MW_COLS + D] = cvt_bf
    cb[0:A, MW_COLS + D :] = ones
    in_maps = []
    for c in range(N_CORES):
        # maskt[p, 4*s + j] = mask[c, l] with l = 512s + 128j + p
        mf = mask[c].astype(np.float32)
        maskt = np.ascontiguousarray(mf.reshape(L // P, P).T)
        cf = np.zeros((P, L // P + 1), dtype=np.float32)
        cf[:, 0 : L // P] = maskt
        cf[0:A, L // P] = biasT[:, 0]
        # x packed as fp8 d-pairs in u16 units: [L, D/2]
        x8 = np.ascontiguousarray(x[c].astype(f8))  # [L, D]
        xp = x8.view(np.uint16).view(bf)  # [L, D/2]
        in_maps.append(
            {
                "x": np.ascontiguousarray(xp),
                "cb": cb,
                "cf": cf,
            }
        )
    return in_maps


def kernel(**inputs):
    x = np.asarray(inputs["x"], dtype=np.float32)
    mask = np.asarray(inputs["mask"])
    W_Q = np.asarray(inputs["W_Q"], dtype=np.float32)
    b_Q = np.asarray(inputs["b_Q"], dtype=np.float32)
    C_K = np.asarray(inputs["C_K"], dtype=np.float32)
    C_V = np.asarray(inputs["C_V"], dtype=np.float32)

    from concourse.bass_utils import run_bass_kernel_spmd

    nc = _get_nc()
    in_maps = _host_inputs(x, mask, W_Q, b_Q, C_K, C_V)
    res = run_bass_kernel_spmd(nc, in_maps, core_ids=list(range(N_CORES)))
    results = res.results if hasattr(res, "results") else res
    out = np.stack(
        [np.asarray(results[c]["out"]).astype(np.float32) for c in range(N_CORES)],
        axis=0,
    )
    return np.ascontiguousarray(out, dtype=np.float32)


# revision 11
# speedup vs baseline: 1.2986x; 1.0297x over previous
"""CoLAttention Trainium2 kernel (8-core data-parallel SPMD).

Computes, per batch b:
    Q   = x @ W_Q.T + b_Q
    A   = softmax((Q @ C_K) / sqrt(D), axis=-1) * mask[..., None]
    out = A @ C_V.T

Algebraic restructure done on host (exact up to fp rounding):
    S    = x @ M + biasT          where  M = (W_Q.T @ C_K)/sqrt(D)  [D, A]
                                          biasT = (b_Q @ C_K)/sqrt(D)  [A]
    out  = (exp(S) @ C_V.T) * (mask / sum_a exp(S))[:, None]
(logits |S| < ~0.3 for these input stats, so no max-subtraction is needed;
the softmax denominator is applied after the second matmul by linearity.
The denominator uses the same rounded exp values as mm2, so the softmax
normalization is exact w.r.t. the rounded weights.)

fp8 DoubleRow mm1: x and M are quantized to e4m3 (M pre-scaled by 512; the
exp activation applies 1/512).  Host packs adjacent-d pairs of fp8 x into
one 16-bit unit, so a single xbar DMA transpose both halves the input
stream bytes and lands the pairs as the DoubleRow k-tile dim:
  SBUF partition p, free (c, 2t+b) = fp8 x[512s+t, 256c+2p+b]
mm1 then runs perf_mode=DoubleRow (contraction 256/instr, 0.5 cyc/row):
  st[a, t] (x512) = sum_c sum_p sum_b mwq[256c+2p+b, a] * x8[l, 256c+2p+b]

Device dataflow per core:
  - cb/cf const loads then 8 per-strip xbar transposes, all on the SP
    queue in emission order (one copy->xbar mode flip; stores are kept
    after the last transpose because the xbar mode is global to the DMA
    fabric)
  - mm1: S.T halves [64, 256] in fp8 DoubleRow, 4 c-chunks each
  - ACT: expT = Exp(S.T * (1/512) + biasT)  (bias + descale fused)
  - rowsums: expT_slice.T @ ones -> [128, 1] psum (fp32 accumulate)
  - mm2: out [128, 512] = expT_slice.T @ C_V.T chunk  (bf16)
  - scale by (mask * 1/rowsum) per-partition into a [128, 4, 1024] strip
    buffer (alternating DVE/ACT), stored with ONE DMA per strip
"""

import math
import os
import sys

import numpy as np

for _p in ("/opt/trn_rl_repo",):
    if _p not in sys.path and os.path.isdir(_p):
        sys.path.insert(0, _p)

B, L, D, A = 8, 4096, 1024, 64
N_CORES = 8
P = 128  # partitions
SL = 512  # l-strip length
NSTRIP = L // SL  # 8
NJ = SL // P  # 4 l-subtiles per strip
NC = D // 256  # 4 DoubleRow contraction chunks (256 d each)
NE = D // SL  # 2 e-chunks of the output row

MW_SCALE = 512.0  # fp8 pre-scale on M; undone in the exp activation

OUT_BF16 = True  # store output as bf16 (halves store traffic)

# cb (bf16-typed) layout: [mwdr bytes (256 bf16 cols) | cvt (1024) | ones (2)]
MW_COLS = 256  # 512 fp8 bytes viewed as 256 bf16 columns
CB_W = MW_COLS + D + 2


def _build_nc():
    import concourse.bass as bass
    import concourse.tile as tile
    from concourse import bacc, mybir

    f32 = mybir.dt.float32
    bf16 = mybir.dt.bfloat16
    f8 = mybir.dt.float8e4
    EXP = mybir.ActivationFunctionType.Exp
    DR = mybir.MatmulPerfMode.DoubleRow
    out_dt = bf16 if OUT_BF16 else f32

    nc = bacc.Bacc(
        "TRN2",
        target_bir_lowering=False,
        debug=False,
        enable_asserts=False,
        num_devices=N_CORES,
    )

    # x packed as fp8 d-pairs in 16-bit units: [L, D/2] "bf16"
    x_ap = nc.dram_tensor("x", [L, D // 2], bf16, kind="ExternalInput").ap()
    cb_ap = nc.dram_tensor("cb", [P, CB_W], bf16, kind="ExternalInput").ap()
    cf_ap = nc.dram_tensor("cf", [P, L // P + 1], f32, kind="ExternalInput").ap()
    out_ap = nc.dram_tensor("out", [L, D], out_dt, kind="ExternalOutput").ap()

    # strip s, subtile j, partition p: l = 512s + 128j + p
    out_r = out_ap.rearrange("(s j p) d -> s p j d", j=NJ, p=P)

    with tile.TileContext(nc) as tc:
        with (
            tc.tile_pool(name="consts", bufs=1) as consts,
            tc.tile_pool(name="xt", bufs=8) as xt_pool,
            tc.tile_pool(name="st", bufs=2, space="PSUM") as st_pool,
            tc.tile_pool(name="rs", bufs=2, space="PSUM") as rs_pool,
            tc.tile_pool(name="op", bufs=4, space="PSUM") as op_pool,
            tc.tile_pool(name="et", bufs=2) as et_pool,
            tc.tile_pool(name="sc", bufs=4) as sc_pool,
            tc.tile_pool(name="ob", bufs=8) as ob_pool,
        ):
            cb_sb = consts.tile([P, CB_W], bf16)
            nc.sync.dma_start(out=cb_sb, in_=cb_ap)
            cf_sb = consts.tile([P, L // P + 1], f32)
            nc.sync.dma_start(out=cf_sb, in_=cf_ap)

            # p-state warm-up: junk matmuls (into recycled st-pool tiles, no
            # extra PSUM banks) keep the PE continuously busy from right
            # after the entry barrier until the first transpose lands,
            # ramping the PE clock 0.65 -> 2.4 GHz before real work.
            wu_sb = consts.tile([P, SL // 2], bf16)
            nc.vector.memset(wu_sb, 0.5)
            for _ in range(20):
                wu_ps = st_pool.tile([A, SL // 2], f32, tag="st")
                nc.tensor.matmul(
                    wu_ps, lhsT=wu_sb[:, 0:A], rhs=wu_sb, start=True, stop=True
                )
            # mwdr[p, c, b, a] = fp8(512 * M[256c + 2p + b, a])
            mwdr_sb = (
                cb_sb[:, 0:MW_COLS]
                .bitcast(f8)
                .rearrange("p (c b a) -> p c b a", c=NC, b=2)
            )
            cvt_sb = cb_sb[0:A, MW_COLS : MW_COLS + D]
            ones_sb = cb_sb[0:A, MW_COLS + D : MW_COLS + D + 2]
            maskt_sb = cf_sb[:, 0 : L // P]
            bias_sb = cf_sb[0:A, L // P : L // P + 1]

            # Phase 1: all xbar transposes back-to-back on the SP sequencer
            # (a single xbar-mode phase -> no per-strip mode-switch drains):
            #   xt[p, c, t] = xpair[512s + t, 128c + p]
            xts = []
            t_insts = []
            for s in range(NSTRIP):
                xt_t = xt_pool.tile([P, NC, SL], bf16, tag="xt")
                ti = nc.sync.dma_start(
                    out=xt_t,
                    in_=x_ap[s * SL : (s + 1) * SL, :],
                    transpose=True,
                )
                xts.append(xt_t)
                t_insts.append(ti)

            for s in range(NSTRIP):
                xt8 = xts[s].bitcast(f8)  # [P, NC, 2*SL] fp8
                # mm1: S.T halves [64, 256], fp8 DoubleRow, accumulated
                # over 4 c-chunks (contraction 256 d per instruction)
                sts = []
                for h in range(2):
                    sth = st_pool.tile([A, SL // 2], f32, tag="st")
                    sts.append(sth)
                for c in range(NC):
                    full = xt8[:, c].rearrange("p (l b) -> p b l", b=2)
                    for h in range(2):
                        nc.tensor.matmul(
                            sts[h],
                            lhsT=mwdr_sb[:, c],
                            rhs=full[:, :, h * (SL // 2) : (h + 1) * (SL // 2)],
                            start=(c == 0),
                            stop=(c == NC - 1),
                            perf_mode=DR,
                        )

                # expT = exp(S.T/512 + bias)
                et = et_pool.tile([A, SL], bf16, tag="et")
                for h in range(2):
                    nc.scalar.activation(
                        et[:, h * (SL // 2) : (h + 1) * (SL // 2)],
                        sts[h],
                        EXP,
                        bias=bias_sb,
                        scale=1.0 / MW_SCALE,
                    )

                ob = ob_pool.tile([P, NJ, D], out_dt, tag="ob")
                for j in range(NJ):
                    lcol = s * NJ + j  # global l-subtile index (0..31)
                    rs = rs_pool.tile([P, 1], f32, tag="rs")
                    nc.tensor.matmul(
                        rs,
                        lhsT=et[:, j * P : (j + 1) * P],
                        rhs=ones_sb[:, 0:1],
                        start=True,
                        stop=True,
                    )
                    sc = sc_pool.tile([P, 1], f32, tag="sc")
                    nc.vector.reciprocal(sc, rs[:, 0:1])
                    scm = sc_pool.tile([P, 1], f32, tag="scm")
                    nc.gpsimd.tensor_mul(scm, sc, maskt_sb[:, lcol : lcol + 1])

                    for e in range(NE):
                        op = op_pool.tile([P, SL], f32, tag="op")
                        nc.tensor.matmul(
                            op,
                            lhsT=et[:, j * P : (j + 1) * P],
                            rhs=cvt_sb[:, e * SL : (e + 1) * SL],
                            start=True,
                            stop=True,
                        )
                        dst = ob[:, j, e * SL : (e + 1) * SL]
                        if (j * NE + e) % 2:
                            nc.scalar.mul(dst, op, scm)
                        else:
                            nc.vector.tensor_scalar_mul(dst, op, scm)
                # stores per strip (interleaved rows via strided AP), ordered
                # after the last transpose to avoid xbar-mode flips; the last
                # strip stores per-j so the final drain is only ~256KB
                if s < NSTRIP - 1:
                    st_is = [nc.sync.dma_start(out=out_r[s], in_=ob)]
                else:
                    st_is = [
                        nc.sync.dma_start(out=out_r[s][:, j_], in_=ob[:, j_])
                        for j_ in range(NJ)
                    ]
                for st_i in st_is:
                    tile.add_dep_helper(
                        st_i.ins, t_insts[-1].ins,
                        reason="keep copy-mode stores after the xbar phase",
                    )

    nc.compile()
    return nc


_NC_CACHE = None


def _get_nc():
    global _NC_CACHE
    if _NC_CACHE is None:
        _NC_CACHE = _build_nc()
    return _NC_CACHE


def _host_inputs(x, mask, W_Q, b_Q, C_K, C_V):
    """Per-core input maps for run_bass_kernel_spmd."""
    import ml_dtypes

    bf = ml_dtypes.bfloat16
    f8 = ml_dtypes.float8_e4m3fn
    inv_sqrt_d = np.float32(1.0 / math.sqrt(D))
    mw = (W_Q.T.astype(np.float32) @ C_K.astype(np.float32)) * inv_sqrt_d
    mwq = np.ascontiguousarray((mw * MW_SCALE).astype(f8))  # [D, A]
    # mwdr[p, c, b, a] = mwq[256c + 2p + b, a]
    mwdr = mwq.reshape(NC, P, 2, A).transpose(1, 0, 2, 3)
    mwdr_b = (
        np.ascontiguousarray(mwdr).reshape(P, 2 * NC * A).view(np.uint16).view(bf)
    )  # [P, 256]
    cvt_bf = np.ascontiguousarray(C_V.T.astype(bf))  # [A, D]
    biasT = ((b_Q.astype(np.float32) @ C_K.astype(np.float32)) * inv_sqrt_d).reshape(
        A, 1
    )
    biasT = np.ascontiguousarray(biasT, dtype=np.float32)
    ones = np.ones((A, 2), dtype=bf)

    cb = np.zeros((P, CB_W), dtype=bf)
    cb[:, 0:MW_COLS] = mwdr_b
    cb[0:A, MW_COLS : MW_COLS + D] = cvt_bf
    cb[0:A, MW_COLS + D :] = ones
    in_maps = []
    for c in range(N_CORES):
        # maskt[p, 4*s + j] = mask[c, l] with l = 512s + 128j + p
        mf = mask[c].astype(np.float32)
        maskt = np.ascontiguousarray(mf.reshape(L // P, P).T)
        cf = np.zeros((P, L // P + 1), dtype=np.float32)
        cf[:, 0 : L // P] = maskt
        cf[0:A, L // P] = biasT[:, 0]
        # x packed as fp8 d-pairs in u16 units: [L, D/2]
        x8 = np.ascontiguousarray(x[c].astype(f8))  # [L, D]
        xp = x8.view(np.uint16).view(bf)  # [L, D/2]
        in_maps.append(
            {
                "x": np.ascontiguousarray(xp),
                "cb": cb,
                "cf": cf,
            }
        )
    return in_maps


def kernel(**inputs):
    x = np.asarray(inputs["x"], dtype=np.float32)
    mask = np.asarray(inputs["mask"])
    W_Q = np.asarray(inputs["W_Q"], dtype=np.float32)
    b_Q = np.asarray(inputs["b_Q"], dtype=np.float32)
    C_K = np.asarray(inputs["C_K"], dtype=np.float32)
    C_V = np.asarray(inputs["C_V"], dtype=np.float32)

    from concourse.bass_utils import run_bass_kernel_spmd

    nc = _get_nc()
    in_maps = _host_inputs(x, mask, W_Q, b_Q, C_K, C_V)
    res = run_bass_kernel_spmd(nc, in_maps, core_ids=list(range(N_CORES)))
    results = res.results if hasattr(res, "results") else res
    out = np.stack(
        [np.asarray(results[c]["out"]).astype(np.float32) for c in range(N_CORES)],
        axis=0,
    )
    return np.ascontiguousarray(out, dtype=np.float32)
